# revision 36
# baseline (speedup 1.0000x reference)
"""Multi-head causal attention (B=4, S=2048, H=16, D=64) on 8 trn2 cores.

Sharding: core c -> (batch b = c//2, half = c%2). Each core computes the
full K/V projections for its batch and attention + output projection for
a zigzag set of 4 query chunks of 256 rows ({0,3,4,7} or {1,2,5,6}) so
that the causal-attention work per core is identical in structure
(uniform SPMD program); per-core differences are carried entirely by the
input data (query rows, binary keep-masks). No collectives needed: each
core owns disjoint output rows.

Perf structure:
- Causal masking is a binary bf16 keep-mask multiplied into ex after the
  exp (DVE/GPSIMD alternating), so the exp chain never waits on a mask
  op and the padded k-blocks are zeroed exactly.
- Input transposes are batched 4 source tiles deep per destination tile
  so each PSUM->SBUF eviction moves [128, 512]; evictions alternate
  ACT/DVE (GPSIMD cannot touch PSUM); the f32->bf16 input conversion
  runs on GPSIMD.
- Phases are software-pipelined per 512-row chunk: K and Q projections
  consume each chunk's transposes as they land; V-projection chunk g
  is immediately followed by attention slot s=g (slot s only reads
  k-blocks < 4(s+1) = sched[s]), so the exp stream overlaps the
  remaining V-projection matmuls instead of serializing after them.
"""

import numpy as np
import ml_dtypes

import concourse.bass as bass
import concourse.mybir as mybir
from concourse.tile import TileContext
from concourse.bass_utils import run_bass_kernel_spmd
from concourse.masks import make_identity

F32 = mybir.dt.float32
BF16 = mybir.dt.bfloat16

B, S, H, D = 4, 2048, 16, 64
DM = H * D           # 1024
QCH = 256            # query chunk rows
NCH = S // QCH       # 8 global chunks per batch
QROWS = S // 2       # query rows per core (1024)
KB = 128             # k block rows
SCHED = (4, 8, 12, 16)   # padded k-extent (in KB blocks) per local slot
GMAP = ((0, 3, 4, 7), (1, 2, 5, 6))  # global chunk per (half, slot)

N_CORES = 8


def _split_excess_waits(nc):
    """walrus on this stack accepts at most ONE semaphore wait per
    instruction; Tile emits more on drains/branches/etc. Move excess
    waits onto preceding same-engine nops (semantically identical: the
    engine blocks on the nops first)."""
    for f in nc.m.functions:
        for bb in f.blocks:
            new_instrs = []
            for ins in bb.instructions:
                si = ins.sync_info
                if si is not None and si.on_wait is not None and len(si.on_wait) > 1:
                    waits = list(si.on_wait)
                    extra, keep = waits[:-1], waits[-1:]
                    for i, w in enumerate(extra):
                        new_instrs.append(mybir.InstNoOp(
                            name=f"{ins.name}-ws{i}", engine=ins.engine,
                            ins=[], outs=[],
                            sync_info=mybir.SyncInfo(on_wait=[w], on_update=[])))
                    ins.sync_info = mybir.SyncInfo(on_wait=keep,
                                                   on_update=list(si.on_update))
                new_instrs.append(ins)
            bb.instructions[:] = new_instrs


def _load_chunk_transposed(nc, pool, psum, ident, src_dram, g4, name,
                           tr_bufs=3, first=False, conv_eng=None):
    """Load 4 source tiles (rows 512*g4..512*g4+511) of src [*, DM] f32,
    convert to bf16 on GPSIMD, PE-transpose into 8 chunk tiles
    [128, 512] (tile j = dm rows 128j..., cols = the 4 source tiles).
    Evictions [128,512] alternate ACT/DVE."""
    tts = []
    bfvs = []
    for k in range(4):
        st = 4 * g4 + k
        raw = pool.tile([128, DM], F32, tag=f"{name}_raw", bufs=6, name="raw")
        eng = nc.sync if st % 2 == 0 else nc.scalar
        eng.dma_start(raw[:], src_dram[st * 128:(st + 1) * 128, :])
        bfv = pool.tile([128, DM], BF16, tag=f"{name}_bf", bufs=6, name="bfv")
        (conv_eng or nc.gpsimd).tensor_copy(bfv[:], raw[:])
        bfvs.append(bfv)
    for j in range(8):
        tt = pool.tile([128, 512], BF16, tag=f"{name}Tc{j}", bufs=2,
                       name=f"{name}Tc{j}")
        tp = psum.tile([128, 512], BF16, tag=f"{name}_tr", bufs=tr_bufs,
                       name="tp")
        for k in range(4):
            nc.tensor.transpose(
                tp[:, k * 128:(k + 1) * 128],
                bfvs[k][:, j * 128:(j + 1) * 128], ident[:])
        if j % 2 == 0:
            nc.scalar.copy(tt[:], tp[:])
        else:
            nc.vector.tensor_copy(tt[:], tp[:])
        tts.append(tt)
    return tts


def build_mha(masking: bool, repeat: int = 1, mask_gps: bool = False,
              fine: bool = True, gps_conv: bool = True):
    nc = bass.Bass()

    q_in = nc.dram_tensor("q_in", [QROWS, DM], F32, kind="ExternalInput")
    k_in = nc.dram_tensor("k_in", [S, DM], F32, kind="ExternalInput")
    v_in = nc.dram_tensor("v_in", [S, DM], F32, kind="ExternalInput")
    wqt = nc.dram_tensor("wqt", [DM, DM], BF16, kind="ExternalInput")
    wkt = nc.dram_tensor("wkt", [DM, DM], BF16, kind="ExternalInput")
    wvt = nc.dram_tensor("wvt", [DM, DM], BF16, kind="ExternalInput")
    wot = nc.dram_tensor("wot", [DM, DM], BF16, kind="ExternalInput")
    bq2 = nc.dram_tensor("bq2", [128, 8], F32, kind="ExternalInput")
    bk2 = nc.dram_tensor("bk2", [128, 8], F32, kind="ExternalInput")
    bvr = nc.dram_tensor("bvr", [1, DM], BF16, kind="ExternalInput")
    bor = nc.dram_tensor("bor", [1, DM], BF16, kind="ExternalInput")
    msk = nc.dram_tensor("msk", [4, 128, 1024], BF16, kind="ExternalInput")
    out = nc.dram_tensor("out", [QROWS, DM], F32, kind="ExternalOutput")

    sched = SCHED if masking else (16, 16, 16, 16)

    for _rep in range(repeat):
      with TileContext(nc) as tc:
        with tc.tile_pool(name="persist", bufs=1) as pp:
            # ---- constants ----
            bq_sb = pp.tile([128, 8], F32, tag="bq")
            bk_sb = pp.tile([128, 8], F32, tag="bk")
            nc.scalar.dma_start(bq_sb[:], bq2[:])
            nc.scalar.dma_start(bk_sb[:], bk2[:])
            bv_sb = pp.tile([1, DM], BF16, tag="bv")
            nc.scalar.dma_start(bv_sb[:], bvr[:])
            bo_sb = pp.tile([1, DM], BF16, tag="bo")
            nc.scalar.dma_start(bo_sb[:], bor[:])
            ones_sb = pp.tile([1, 128], BF16, tag="ones")
            nc.vector.memset(ones_sb[:], 1.0)
            ident = pp.tile([128, 128], BF16, tag="ident")
            make_identity(nc, ident[:])
            mask_sb = []
            if masking:
                for s in range(4):
                    mt = pp.tile([128, 1024], BF16, tag=f"msk{s}")
                    nc.scalar.dma_start(mt[:], msk[s])
                    mask_sb.append(mt)

            # ---- persistent activation storage ----
            kT = [pp.tile([128, S], BF16, tag=f"kT{j}", name=f"kT{j}") for j in range(8)]
            qT = [pp.tile([128, QROWS], BF16, tag=f"qT{j}", name=f"qT{j}") for j in range(8)]
            v_sb = [pp.tile([128, H * (D + 1)], BF16, tag=f"v{t}", name=f"v{t}")
                    for t in range(S // 128)]
            attn = [pp.tile([128, DM], BF16, tag=f"attn{t}", name=f"attn{t}")
                    for t in range(QROWS // 128)]

            # ---- K + Q projections, chunk-pipelined ----
            with (tc.tile_pool(name="kqstage", bufs=1) as ksp,
                  tc.tile_pool(name="kqps", bufs=2, space="PSUM") as kps):
                wk = [ksp.tile([128, DM], BF16, tag=f"wk{j}", name=f"wk{j}")
                      for j in range(8)]
                wq = [ksp.tile([128, DM], BF16, tag=f"wq{j}", name=f"wq{j}")
                      for j in range(8)]
                for j in range(8):
                    nc.sync.dma_start(wk[j][:], wkt[j * 128:(j + 1) * 128, :])
                    nc.scalar.dma_start(wq[j][:], wqt[j * 128:(j + 1) * 128, :])
                for g4 in range(4):
                    keyTc = _load_chunk_transposed(
                        nc, ksp, kps, ident, k_in, g4, "kq", first=(g4 == 0),
                        conv_eng=None if gps_conv else nc.vector)
                    for i in range(8):
                        p = kps.tile([128, 512], F32, tag="proj", bufs=2)
                        for j in range(8):
                            nc.tensor.matmul(
                                p[:], wk[j][:, i * 128:(i + 1) * 128],
                                keyTc[j][:],
                                start=(j == 0), stop=(j == 7))
                        nc.vector.tensor_scalar_add(
                            kT[i][:, g4 * 512:(g4 + 1) * 512], p[:],
                            bk_sb[:, i:i + 1])
                for g4 in range(2):
                    quTc = _load_chunk_transposed(
                        nc, ksp, kps, ident, q_in, g4, "kq",
                        conv_eng=None if gps_conv else nc.vector)
                    for i in range(8):
                        p = kps.tile([128, 512], F32, tag="proj", bufs=2)
                        for j in range(8):
                            nc.tensor.matmul(
                                p[:], wq[j][:, i * 128:(i + 1) * 128],
                                quTc[j][:],
                                start=(j == 0), stop=(j == 7))
                        nc.vector.tensor_scalar_add(
                            qT[i][:, g4 * 512:(g4 + 1) * 512], p[:],
                            bq_sb[:, i:i + 1])

            # ---- V projection staggered with attention slots ----
            with (
                tc.tile_pool(name="vstage", bufs=1) as vsp,
                tc.tile_pool(name="vps", bufs=1, space="PSUM") as vps,
                tc.tile_pool(name="scores", bufs=2, space="PSUM") as scp,
                tc.tile_pool(name="avp", bufs=1, space="PSUM") as avp,
                tc.tile_pool(name="expp", bufs=3) as exp_pool,
                tc.tile_pool(name="recp", bufs=4) as rec_pool,
            ):
                wv = [vsp.tile([128, DM], BF16, tag=f"wv{j}", name=f"wv{j}")
                      for j in range(8)]
                wo = [vsp.tile([128, DM], BF16, tag=f"wo{j}", name=f"wo{j}")
                      for j in range(8)]
                attnT = [vsp.tile([128, QROWS], BF16, tag=f"attnT{j}",
                                  name=f"attnT{j}") for j in range(8)]
                for j in range(8):
                    nc.scalar.dma_start(wv[j][:], wvt[j * 128:(j + 1) * 128, :])
                    nc.sync.dma_start(wo[j][:], wot[j * 128:(j + 1) * 128, :])

                def make_vchunk_pieces(g4, pre_bfvs=None):
                    """V chunk g4 split into 16 emission pieces: 4 loads,
                    8 transpose groups, 4 st projections. Interleaved a few
                    pieces per attention head of the preceding slot so PE
                    alternates fine-grained between scores and V-proj and
                    the exp stream never starves."""
                    bfvs, valTc = [], []
                    if pre_bfvs is not None:
                        bfvs.extend(pre_bfvs)

                    def load(k):
                        st = 4 * g4 + k
                        raw = vsp.tile([128, DM], F32, tag="val_raw",
                                       bufs=4, name="raw")
                        eng = (nc.sync, nc.scalar)[st % 2]
                        eng.dma_start(raw[:],
                                      v_in[st * 128:(st + 1) * 128, :])
                        bfv = vsp.tile([128, DM], BF16, tag="val_bf",
                                       bufs=4, name="bfv")
                        ceng = nc.gpsimd if gps_conv else nc.vector
                        ceng.tensor_copy(bfv[:], raw[:])
                        bfvs.append(bfv)

                    def trans(j):
                        tt = vsp.tile([128, 512], BF16, tag=f"valTc{j}",
                                      bufs=2, name=f"valTc{j}")
                        tp = vps.tile([128, 512], BF16, tag="val_tr",
                                      bufs=1, name="tp")
                        for k in range(4):
                            nc.tensor.transpose(
                                tp[:, k * 128:(k + 1) * 128],
                                bfvs[k][:, j * 128:(j + 1) * 128], ident[:])
                        nc.vector.tensor_copy(tt[:], tp[:])
                        valTc.append(tt)

                    def proj(k):
                        st = 4 * g4 + k
                        v3 = v_sb[st].rearrange("p (h x) -> p h x", x=D + 1)
                        nc.vector.memset(v3[:, :, 64:65], 1.0)
                        for c in range(2):
                            p = vps.tile([128, 512], F32, tag="vproj",
                                         bufs=1)
                            for j in range(8):
                                nc.tensor.matmul(
                                    p[:], valTc[j][:, k * 128:(k + 1) * 128],
                                    wv[j][:, c * 512:(c + 1) * 512],
                                    start=(j == 0), stop=False)
                            nc.tensor.matmul(
                                p[:], ones_sb[:],
                                bv_sb[:, c * 512:(c + 1) * 512],
                                start=False, stop=True)
                            nc.vector.tensor_copy(
                                v3[:, c * 8:(c + 1) * 8, 0:64], p[:])

                    loads = [] if pre_bfvs is not None else \
                        [(load, k) for k in range(4)]
                    return (loads
                            + [(trans, j) for j in range(8)]
                            + [(proj, k) for k in range(4)])

                def make_outproj_pieces(t2):
                    """Output projection for attn tiles 2*t2, 2*t2+1
                    (finished by slot t2), split into 10 pieces and
                    interleaved into later slots (tail for t2=3)."""
                    def trans(j):
                        tp = vps.tile([128, 512], BF16, tag="val_tr",
                                      bufs=1, name="tp")
                        for k in range(2):
                            nc.tensor.transpose(
                                tp[:, k * 128:(k + 1) * 128],
                                attn[2 * t2 + k][:, j * 128:(j + 1) * 128],
                                ident[:])
                        nc.vector.tensor_copy(
                            attnT[j][:, t2 * 256:(t2 + 1) * 256],
                            tp[:, 0:256])

                    def oproj(t):
                        ot = vsp.tile([128, DM], F32, tag="ot", bufs=1,
                                      name="ot")
                        for c in range(2):
                            p = vps.tile([128, 512], F32, tag="vproj",
                                         bufs=1)
                            for j in range(8):
                                nc.tensor.matmul(
                                    p[:], attnT[j][:, t * 128:(t + 1) * 128],
                                    wo[j][:, c * 512:(c + 1) * 512],
                                    start=(j == 0), stop=False)
                            nc.tensor.matmul(
                                p[:], ones_sb[:],
                                bo_sb[:, c * 512:(c + 1) * 512],
                                start=False, stop=True)
                            nc.vector.tensor_copy(
                                ot[:, c * 512:(c + 1) * 512], p[:])
                        nc.sync.dma_start(out[t * 128:(t + 1) * 128, :],
                                          ot[:])

                    return ([(trans, j) for j in range(8)]
                            + [(oproj, 2 * t2 + k) for k in range(2)])

                # prologue: V chunk 0 emitted whole (all chunks when not
                # masking: slot 0 then reads every k block)
                n_pro = 1 if masking else 4
                for g4 in range(n_pro):
                    for fn, a in make_vchunk_pieces(g4):
                        fn(a)
                for s in range(4):
                    G = sched[s] // 4
                    pieces = (make_vchunk_pieces(s + 1)
                              if masking and s < 3 else [])
                    if s == 2:
                        pieces = pieces + make_outproj_pieces(0)
                    elif s == 3:
                        pieces = (pieces + make_outproj_pieces(1)
                                  + make_outproj_pieces(2))
                    if not fine:
                        for fn, a in pieces:
                            fn(a)
                        pieces = []
                    for h in range(H):
                        ht, ho = h // 2, (h % 2) * 64
                        av = [avp.tile([128, 65], F32, tag=f"av{q2}",
                                       name=f"av{q2}")[:]
                              for q2 in range(2)]
                        for g in range(G):
                            sc = scp.tile([128, 1024], F32, tag="sc")
                            for jj in range(4):
                                kb = 4 * g + jj
                                nc.tensor.matmul(
                                    sc[:, jj * 256:(jj + 1) * 256],
                                    kT[ht][ho:ho + 64, kb * 128:(kb + 1) * 128],
                                    qT[ht][ho:ho + 64, s * 256:(s + 1) * 256],
                                    start=True, stop=True)
                            ex = exp_pool.tile([128, 1024], BF16, tag="ex")
                            nc.scalar.activation(
                                ex[:], sc[:],
                                mybir.ActivationFunctionType.Exp, scale=0.125)
                            if masking and g == G - 1:
                                meng = (nc.vector if (h % 2 == 0 or
                                        not mask_gps) else nc.gpsimd)
                                meng.tensor_mul(ex[:], ex[:], mask_sb[s][:])
                            for jj in range(4):
                                for q2 in range(2):
                                    nc.tensor.matmul(
                                        av[q2][:, :],
                                        ex[:, jj * 256 + q2 * 128:
                                           jj * 256 + q2 * 128 + 128],
                                        v_sb[4 * g + jj][:, 65 * h:65 * h + 65],
                                        start=(g == 0 and jj == 0),
                                        stop=(g == G - 1 and jj == 3))
                        for q2 in range(2):
                            rec = rec_pool.tile([128, 1], F32, tag="rec")
                            nc.vector.reciprocal(rec[:], av[q2][:, 64:65])
                            nc.vector.tensor_scalar_mul(
                                attn[2 * s + q2][:, 64 * h:64 * h + 64],
                                av[q2][:, 0:64], rec[:])
                        lo = h * len(pieces) // H
                        hi = (h + 1) * len(pieces) // H
                        for fn, a in pieces[lo:hi]:
                            fn(a)
                # tail: output projection for slot 3's tiles
                for fn, a in make_outproj_pieces(3):
                    fn(a)

    _split_excess_waits(nc)
    return nc


def _build_masks(half: int) -> np.ndarray:
    """Binary keep-mask (1=keep, 0=masked) for the LAST 4-kb group of each
    slot, multiplied into ex post-exp: [4, 128, 1024] bf16, free dim =
    kb_local*256 + dq."""
    m = np.zeros((4, 128, 1024), np.float32)
    dk = np.arange(128)[:, None]
    dq = np.arange(256)[None, :]
    for s in range(4):
        L = SCHED[s]
        g = GMAP[half][s]
        for jj in range(4):
            kb = L - 4 + jj
            kg = kb * 128 + dk
            qg = g * 256 + dq
            m[s, :, jj * 256:(jj + 1) * 256] = np.where(kg <= qg, 1.0, 0.0)
    return m.astype(ml_dtypes.bfloat16)


_CACHE = {}


def kernel(query, key, value, Wq, bq, Wk, bk, Wv, bv, Wo, bo, masking):
    query = np.asarray(query, np.float32)
    key = np.asarray(key, np.float32)
    value = np.asarray(value, np.float32)
    masking = bool(int(np.asarray(masking)))

    bf = ml_dtypes.bfloat16
    wqt = np.ascontiguousarray(np.asarray(Wq, np.float32).T).astype(bf)
    wkt = np.ascontiguousarray(np.asarray(Wk, np.float32).T).astype(bf)
    wvt = np.ascontiguousarray(np.asarray(Wv, np.float32).T).astype(bf)
    wot = np.ascontiguousarray(np.asarray(Wo, np.float32).T).astype(bf)
    bq2 = np.ascontiguousarray(np.asarray(bq, np.float32).reshape(8, 128).T)
    bk2 = np.ascontiguousarray(np.asarray(bk, np.float32).reshape(8, 128).T)
    bvr = np.asarray(bv, np.float32).reshape(1, DM).astype(bf)
    bor = np.asarray(bo, np.float32).reshape(1, DM).astype(bf)

    if masking not in _CACHE:
        _CACHE[masking] = build_mha(masking)
    nc = _CACHE[masking]
    in_maps = make_in_maps(query, key, value, wqt, wkt, wvt, wot,
                           bq2, bk2, bvr, bor, masking)
    res = run_bass_kernel_spmd(nc, in_maps, list(range(N_CORES)))
    return gather_out([r["out"] for r in res.results], masking)


def make_in_maps(query, key, value, wqt, wkt, wvt, wot, bq2, bk2, bvr, bor,
                 masking):
    in_maps = []
    for c in range(N_CORES):
        b, half = c // 2, c % 2
        gmap = GMAP[half] if masking else (
            (0, 1, 2, 3) if half == 0 else (4, 5, 6, 7))
        qch = query[b].reshape(NCH, QCH, DM)
        q_sh = np.ascontiguousarray(
            np.concatenate([qch[g] for g in gmap], axis=0))
        in_maps.append({
            "q_in": q_sh, "k_in": key[b], "v_in": value[b],
            "wqt": wqt, "wkt": wkt, "wvt": wvt, "wot": wot,
            "bq2": bq2, "bk2": bk2, "bvr": bvr, "bor": bor,
            "msk": _build_masks(half) if masking else
                   np.zeros((4, 128, 1024), ml_dtypes.bfloat16),
        })

    return in_maps


def gather_out(core_outs, masking):
    out = np.empty((B, S, DM), np.float32)
    for c in range(N_CORES):
        b, half = c // 2, c % 2
        gmap = GMAP[half] if masking else (
            (0, 1, 2, 3) if half == 0 else (4, 5, 6, 7))
        o = np.asarray(core_outs[c]).reshape(4, QCH, DM)
        for s, g in enumerate(gmap):
            out[b, g * QCH:(g + 1) * QCH, :] = o[s]
    return out


# revision 43
# speedup vs baseline: 1.1935x; 1.1935x over previous
"""Multi-head causal attention (B=4, S=2048, H=16, D=64) on 8 trn2 cores.

Sharding: core c -> (batch b = c//2, half = c%2). Each core computes the
full K/V projections for its batch and attention + output projection for
a zigzag set of 4 query chunks of 256 rows ({0,3,4,7} or {1,2,5,6}) so
that the causal-attention work per core is identical in structure
(uniform SPMD program); per-core differences are carried entirely by the
input data (query rows, binary keep-masks). No collectives needed: each
core owns disjoint output rows.

Perf structure:
- Causal masking is a binary bf16 keep-mask multiplied into ex after the
  exp (DVE/GPSIMD alternating), so the exp chain never waits on a mask
  op and the padded k-blocks are zeroed exactly.
- Input transposes are batched 4 source tiles deep per destination tile
  so each PSUM->SBUF eviction moves [128, 512]; evictions alternate
  ACT/DVE (GPSIMD cannot touch PSUM); the f32->bf16 input conversion
  runs on GPSIMD.
- Phases are software-pipelined per 512-row chunk: K and Q projections
  consume each chunk's transposes as they land; V-projection chunk g
  is immediately followed by attention slot s=g (slot s only reads
  k-blocks < 4(s+1) = sched[s]), so the exp stream overlaps the
  remaining V-projection matmuls instead of serializing after them.
"""

import numpy as np
import ml_dtypes

import concourse.bass as bass
import concourse.mybir as mybir
from concourse.tile import TileContext
from concourse.bass_utils import run_bass_kernel_spmd
from concourse.masks import make_identity

F32 = mybir.dt.float32
BF16 = mybir.dt.bfloat16

B, S, H, D = 4, 2048, 16, 64
DM = H * D           # 1024
QCH = 256            # query chunk rows
NCH = S // QCH       # 8 global chunks per batch
QROWS = S // 2       # query rows per core (1024)
KB = 128             # k block rows
SCHED = (4, 8, 12, 16)   # padded k-extent (in KB blocks) per local slot
GMAP = ((0, 3, 4, 7), (1, 2, 5, 6))  # global chunk per (half, slot)

N_CORES = 8


def _split_excess_waits(nc):
    """walrus on this stack accepts at most ONE semaphore wait per
    instruction; Tile emits more on drains/branches/etc. Move excess
    waits onto preceding same-engine nops (semantically identical: the
    engine blocks on the nops first)."""
    for f in nc.m.functions:
        for bb in f.blocks:
            new_instrs = []
            for ins in bb.instructions:
                si = ins.sync_info
                if si is not None and si.on_wait is not None and len(si.on_wait) > 1:
                    waits = list(si.on_wait)
                    extra, keep = waits[:-1], waits[-1:]
                    for i, w in enumerate(extra):
                        new_instrs.append(mybir.InstNoOp(
                            name=f"{ins.name}-ws{i}", engine=ins.engine,
                            ins=[], outs=[],
                            sync_info=mybir.SyncInfo(on_wait=[w], on_update=[])))
                    ins.sync_info = mybir.SyncInfo(on_wait=keep,
                                                   on_update=list(si.on_update))
                new_instrs.append(ins)
            bb.instructions[:] = new_instrs


def _load_chunk_transposed(nc, pool, psum, ident, src_dram, g4, name,
                           tr_bufs=3, first=False, conv_eng=None):
    """Load 4 source tiles (rows 512*g4..512*g4+511) of src [*, DM] f32,
    convert to bf16 on GPSIMD, PE-transpose into 8 chunk tiles
    [128, 512] (tile j = dm rows 128j..., cols = the 4 source tiles).
    Evictions [128,512] alternate ACT/DVE."""
    tts = []
    bfvs = []
    for k in range(4):
        st = 4 * g4 + k
        raw = pool.tile([128, DM], F32, tag=f"{name}_raw", bufs=6, name="raw")
        eng = nc.sync if st % 2 == 0 else nc.scalar
        eng.dma_start(raw[:], src_dram[st * 128:(st + 1) * 128, :])
        bfv = pool.tile([128, DM], BF16, tag=f"{name}_bf", bufs=6, name="bfv")
        (conv_eng or nc.gpsimd).tensor_copy(bfv[:], raw[:])
        bfvs.append(bfv)
    for j in range(8):
        tt = pool.tile([128, 512], BF16, tag=f"{name}Tc{j}", bufs=2,
                       name=f"{name}Tc{j}")
        tp = psum.tile([128, 512], BF16, tag=f"{name}_tr", bufs=tr_bufs,
                       name="tp")
        for k in range(4):
            nc.tensor.transpose(
                tp[:, k * 128:(k + 1) * 128],
                bfvs[k][:, j * 128:(j + 1) * 128], ident[:])
        if j % 2 == 0:
            nc.scalar.copy(tt[:], tp[:])
        else:
            nc.vector.tensor_copy(tt[:], tp[:])
        tts.append(tt)
    return tts


def build_mha(masking: bool, repeat: int = 1, mask_gps: bool = False,
              fine: bool = True, gps_conv: bool = True):
    nc = bass.Bass()

    q_in = nc.dram_tensor("q_in", [QROWS, DM], F32, kind="ExternalInput")
    k_in = nc.dram_tensor("k_in", [S, DM], F32, kind="ExternalInput")
    v_in = nc.dram_tensor("v_in", [S, DM], F32, kind="ExternalInput")
    wqt = nc.dram_tensor("wqt", [DM, DM], BF16, kind="ExternalInput")
    wkt = nc.dram_tensor("wkt", [DM, DM], BF16, kind="ExternalInput")
    wvt = nc.dram_tensor("wvt", [DM, DM], BF16, kind="ExternalInput")
    wot = nc.dram_tensor("wot", [DM, DM], BF16, kind="ExternalInput")
    bq2 = nc.dram_tensor("bq2", [128, 8], F32, kind="ExternalInput")
    bk2 = nc.dram_tensor("bk2", [128, 8], F32, kind="ExternalInput")
    bvr = nc.dram_tensor("bvr", [1, DM], BF16, kind="ExternalInput")
    bor = nc.dram_tensor("bor", [1, DM], BF16, kind="ExternalInput")
    msk = nc.dram_tensor("msk", [4, 128, 1024], BF16, kind="ExternalInput")
    out = nc.dram_tensor("out", [QROWS, DM], F32, kind="ExternalOutput")

    sched = SCHED if masking else (16, 16, 16, 16)

    for _rep in range(repeat):
      with TileContext(nc) as tc:
        with tc.tile_pool(name="persist", bufs=1) as pp:
            # ---- constants ----
            bq_sb = pp.tile([128, 8], F32, tag="bq")
            bk_sb = pp.tile([128, 8], F32, tag="bk")
            nc.scalar.dma_start(bq_sb[:], bq2[:])
            nc.scalar.dma_start(bk_sb[:], bk2[:])
            bv_sb = pp.tile([1, DM], BF16, tag="bv")
            nc.scalar.dma_start(bv_sb[:], bvr[:])
            bo_sb = pp.tile([1, DM], BF16, tag="bo")
            nc.scalar.dma_start(bo_sb[:], bor[:])
            ones_sb = pp.tile([1, 128], BF16, tag="ones")
            nc.vector.memset(ones_sb[:], 1.0)
            ident = pp.tile([128, 128], BF16, tag="ident")
            make_identity(nc, ident[:])
            mask_sb = []
            if masking:
                for s in range(4):
                    mt = pp.tile([128, 1024], BF16, tag=f"msk{s}")
                    nc.scalar.dma_start(mt[:], msk[s])
                    mask_sb.append(mt)

            # ---- persistent activation storage ----
            kT = [pp.tile([128, S], BF16, tag=f"kT{j}", name=f"kT{j}") for j in range(8)]
            qT = [pp.tile([128, QROWS], BF16, tag=f"qT{j}", name=f"qT{j}") for j in range(8)]
            v_sb = [pp.tile([128, H * (D + 1)], BF16, tag=f"v{t}", name=f"v{t}")
                    for t in range(S // 128)]
            attn = [pp.tile([128, DM], BF16, tag=f"attn{t}", name=f"attn{t}")
                    for t in range(QROWS // 128)]

            # ---- K + Q projections, chunk-pipelined ----
            with (tc.tile_pool(name="kqstage", bufs=1) as ksp,
                  tc.tile_pool(name="kqps", bufs=2, space="PSUM") as kps):
                wk = [ksp.tile([128, DM], BF16, tag=f"wk{j}", name=f"wk{j}")
                      for j in range(8)]
                wq = [ksp.tile([128, DM], BF16, tag=f"wq{j}", name=f"wq{j}")
                      for j in range(8)]
                for j in range(8):
                    nc.sync.dma_start(wk[j][:], wkt[j * 128:(j + 1) * 128, :])
                    nc.scalar.dma_start(wq[j][:], wqt[j * 128:(j + 1) * 128, :])
                for g4 in range(4):
                    keyTc = _load_chunk_transposed(
                        nc, ksp, kps, ident, k_in, g4, "kq", first=(g4 == 0),
                        conv_eng=None if gps_conv else nc.vector)
                    for i in range(8):
                        p = kps.tile([128, 512], F32, tag="proj", bufs=2)
                        for j in range(8):
                            nc.tensor.matmul(
                                p[:], wk[j][:, i * 128:(i + 1) * 128],
                                keyTc[j][:],
                                start=(j == 0), stop=(j == 7))
                        nc.vector.tensor_scalar_add(
                            kT[i][:, g4 * 512:(g4 + 1) * 512], p[:],
                            bk_sb[:, i:i + 1])
                for g4 in range(2):
                    quTc = _load_chunk_transposed(
                        nc, ksp, kps, ident, q_in, g4, "kq",
                        conv_eng=None if gps_conv else nc.vector)
                    for i in range(8):
                        p = kps.tile([128, 512], F32, tag="proj", bufs=2)
                        for j in range(8):
                            nc.tensor.matmul(
                                p[:], wq[j][:, i * 128:(i + 1) * 128],
                                quTc[j][:],
                                start=(j == 0), stop=(j == 7))
                        nc.vector.tensor_scalar_add(
                            qT[i][:, g4 * 512:(g4 + 1) * 512], p[:],
                            bq_sb[:, i:i + 1])

            # ---- V projection staggered with attention slots ----
            with (
                tc.tile_pool(name="vstage", bufs=1) as vsp,
                tc.tile_pool(name="vps", bufs=1, space="PSUM") as vps,
                tc.tile_pool(name="scores", bufs=2, space="PSUM") as scp,
                tc.tile_pool(name="avp", bufs=1, space="PSUM") as avp,
                tc.tile_pool(name="expp", bufs=3) as exp_pool,
                tc.tile_pool(name="recp", bufs=4) as rec_pool,
            ):
                wv = [vsp.tile([128, DM], BF16, tag=f"wv{j}", name=f"wv{j}")
                      for j in range(8)]
                wo = [vsp.tile([128, DM], BF16, tag=f"wo{j}", name=f"wo{j}")
                      for j in range(8)]
                attnT = [vsp.tile([128, QROWS], BF16, tag=f"attnT{j}",
                                  name=f"attnT{j}") for j in range(8)]
                for j in range(8):
                    nc.scalar.dma_start(wv[j][:], wvt[j * 128:(j + 1) * 128, :])
                    nc.sync.dma_start(wo[j][:], wot[j * 128:(j + 1) * 128, :])

                def make_vchunk_pieces(g4, pre_bfvs=None):
                    """V chunk g4 split into 16 emission pieces: 4 loads,
                    8 transpose groups, 4 st projections. Interleaved a few
                    pieces per attention head of the preceding slot so PE
                    alternates fine-grained between scores and V-proj and
                    the exp stream never starves."""
                    bfvs, valTc = [], []
                    if pre_bfvs is not None:
                        bfvs.extend(pre_bfvs)

                    def load(k):
                        st = 4 * g4 + k
                        raw = vsp.tile([128, DM], F32, tag="val_raw",
                                       bufs=4, name="raw")
                        eng = (nc.sync, nc.scalar)[st % 2]
                        eng.dma_start(raw[:],
                                      v_in[st * 128:(st + 1) * 128, :])
                        bfv = vsp.tile([128, DM], BF16, tag="val_bf",
                                       bufs=4, name="bfv")
                        ceng = nc.gpsimd if gps_conv else nc.vector
                        ceng.tensor_copy(bfv[:], raw[:])
                        bfvs.append(bfv)

                    def trans(j):
                        tt = vsp.tile([128, 512], BF16, tag=f"valTc{j}",
                                      bufs=2, name=f"valTc{j}")
                        tp = vps.tile([128, 512], BF16, tag="val_tr",
                                      bufs=1, name="tp")
                        for k in range(4):
                            nc.tensor.transpose(
                                tp[:, k * 128:(k + 1) * 128],
                                bfvs[k][:, j * 128:(j + 1) * 128], ident[:])
                        nc.vector.tensor_copy(tt[:], tp[:])
                        valTc.append(tt)

                    def proj(k):
                        st = 4 * g4 + k
                        v3 = v_sb[st].rearrange("p (h x) -> p h x", x=D + 1)
                        nc.vector.memset(v3[:, :, 64:65], 1.0)
                        for c in range(2):
                            p = vps.tile([128, 512], F32, tag="vproj",
                                         bufs=1)
                            for j in range(8):
                                nc.tensor.matmul(
                                    p[:], valTc[j][:, k * 128:(k + 1) * 128],
                                    wv[j][:, c * 512:(c + 1) * 512],
                                    start=(j == 0), stop=False)
                            nc.tensor.matmul(
                                p[:], ones_sb[:],
                                bv_sb[:, c * 512:(c + 1) * 512],
                                start=False, stop=True)
                            nc.vector.tensor_copy(
                                v3[:, c * 8:(c + 1) * 8, 0:64], p[:])

                    loads = [] if pre_bfvs is not None else \
                        [(load, k) for k in range(4)]
                    return (loads
                            + [(trans, j) for j in range(8)]
                            + [(proj, k) for k in range(4)])

                def make_outproj_pieces(t2):
                    """Output projection for attn tiles 2*t2, 2*t2+1
                    (finished by slot t2), split into 10 pieces and
                    interleaved into later slots (tail for t2=3)."""
                    def trans(j):
                        tp = vps.tile([128, 512], BF16, tag="val_tr",
                                      bufs=1, name="tp")
                        for k in range(2):
                            nc.tensor.transpose(
                                tp[:, k * 128:(k + 1) * 128],
                                attn[2 * t2 + k][:, j * 128:(j + 1) * 128],
                                ident[:])
                        nc.vector.tensor_copy(
                            attnT[j][:, t2 * 256:(t2 + 1) * 256],
                            tp[:, 0:256])

                    def oproj(t):
                        ot = vsp.tile([128, DM], F32, tag="ot", bufs=1,
                                      name="ot")
                        for c in range(2):
                            p = vps.tile([128, 512], F32, tag="vproj",
                                         bufs=1)
                            for j in range(8):
                                nc.tensor.matmul(
                                    p[:], attnT[j][:, t * 128:(t + 1) * 128],
                                    wo[j][:, c * 512:(c + 1) * 512],
                                    start=(j == 0), stop=False)
                            nc.tensor.matmul(
                                p[:], ones_sb[:],
                                bo_sb[:, c * 512:(c + 1) * 512],
                                start=False, stop=True)
                            nc.vector.tensor_copy(
                                ot[:, c * 512:(c + 1) * 512], p[:])
                        nc.sync.dma_start(out[t * 128:(t + 1) * 128, :],
                                          ot[:])

                    return ([(trans, j) for j in range(8)]
                            + [(oproj, 2 * t2 + k) for k in range(2)])

                # prologue: V chunk 0 emitted whole (all chunks when not
                # masking: slot 0 then reads every k block)
                n_pro = 1 if masking else 4
                for g4 in range(n_pro):
                    for fn, a in make_vchunk_pieces(g4):
                        fn(a)
                for s in range(4):
                    G = sched[s] // 4
                    pieces = (make_vchunk_pieces(s + 1)
                              if masking and s < 3 else [])
                    if s == 2:
                        pieces = pieces + make_outproj_pieces(0)
                    elif s == 3:
                        pieces = (pieces + make_outproj_pieces(1)
                                  + make_outproj_pieces(2))
                    if not fine:
                        for fn, a in pieces:
                            fn(a)
                        pieces = []
                    for h in range(H):
                        ht, ho = h // 2, (h % 2) * 64
                        av = [avp.tile([128, 65], F32, tag=f"av{q2}",
                                       name=f"av{q2}")[:]
                              for q2 in range(2)]
                        for g in range(G):
                            sc = scp.tile([128, 1024], F32, tag="sc")
                            for jj in range(4):
                                kb = 4 * g + jj
                                nc.tensor.matmul(
                                    sc[:, jj * 256:(jj + 1) * 256],
                                    kT[ht][ho:ho + 64, kb * 128:(kb + 1) * 128],
                                    qT[ht][ho:ho + 64, s * 256:(s + 1) * 256],
                                    start=True, stop=True)
                            ex = exp_pool.tile([128, 1024], BF16, tag="ex")
                            nc.scalar.activation(
                                ex[:], sc[:],
                                mybir.ActivationFunctionType.Exp, scale=0.125)
                            if masking and g == G - 1:
                                meng = (nc.vector if (h % 2 == 0 or
                                        not mask_gps) else nc.gpsimd)
                                meng.tensor_mul(ex[:], ex[:], mask_sb[s][:])
                            for jj in range(4):
                                for q2 in range(2):
                                    nc.tensor.matmul(
                                        av[q2][:, :],
                                        ex[:, jj * 256 + q2 * 128:
                                           jj * 256 + q2 * 128 + 128],
                                        v_sb[4 * g + jj][:, 65 * h:65 * h + 65],
                                        start=(g == 0 and jj == 0),
                                        stop=(g == G - 1 and jj == 3))
                        for q2 in range(2):
                            rec = rec_pool.tile([128, 1], F32, tag="rec")
                            nc.vector.reciprocal(rec[:], av[q2][:, 64:65])
                            nc.vector.tensor_scalar_mul(
                                attn[2 * s + q2][:, 64 * h:64 * h + 64],
                                av[q2][:, 0:64], rec[:])
                        lo = h * len(pieces) // H
                        hi = (h + 1) * len(pieces) // H
                        for fn, a in pieces[lo:hi]:
                            fn(a)
                # tail: output projection for slot 3's tiles
                for fn, a in make_outproj_pieces(3):
                    fn(a)

    _split_excess_waits(nc)
    return nc


def _build_masks(half: int) -> np.ndarray:
    """Binary keep-mask (1=keep, 0=masked) for the LAST 4-kb group of each
    slot, multiplied into ex post-exp: [4, 128, 1024] bf16, free dim =
    kb_local*256 + dq."""
    m = np.zeros((4, 128, 1024), np.float32)
    dk = np.arange(128)[:, None]
    dq = np.arange(256)[None, :]
    for s in range(4):
        L = SCHED[s]
        g = GMAP[half][s]
        for jj in range(4):
            kb = L - 4 + jj
            kg = kb * 128 + dk
            qg = g * 256 + dq
            m[s, :, jj * 256:(jj + 1) * 256] = np.where(kg <= qg, 1.0, 0.0)
    return m.astype(ml_dtypes.bfloat16)


_CACHE = {}


def kernel(query, key, value, Wq, bq, Wk, bk, Wv, bv, Wo, bo, masking):
    query = np.asarray(query, np.float32)
    key = np.asarray(key, np.float32)
    value = np.asarray(value, np.float32)
    masking = bool(int(np.asarray(masking)))

    bf = ml_dtypes.bfloat16
    wqt = np.ascontiguousarray(np.asarray(Wq, np.float32).T).astype(bf)
    wkt = np.ascontiguousarray(np.asarray(Wk, np.float32).T).astype(bf)
    wvt = np.ascontiguousarray(np.asarray(Wv, np.float32).T).astype(bf)
    wot = np.ascontiguousarray(np.asarray(Wo, np.float32).T).astype(bf)
    bq2 = np.ascontiguousarray(np.asarray(bq, np.float32).reshape(8, 128).T)
    bk2 = np.ascontiguousarray(np.asarray(bk, np.float32).reshape(8, 128).T)
    bvr = np.asarray(bv, np.float32).reshape(1, DM).astype(bf)
    bor = np.asarray(bo, np.float32).reshape(1, DM).astype(bf)

    if masking not in _CACHE:
        _CACHE[masking] = build_mha(masking)
    nc = _CACHE[masking]
    in_maps = make_in_maps(query, key, value, wqt, wkt, wvt, wot,
                           bq2, bk2, bvr, bor, masking)
    res = run_bass_kernel_spmd(nc, in_maps, list(range(N_CORES)))
    return gather_out([r["out"] for r in res.results], masking)


def make_in_maps(query, key, value, wqt, wkt, wvt, wot, bq2, bk2, bvr, bor,
                 masking):
    in_maps = []
    for c in range(N_CORES):
        b, half = c // 2, c % 2
        gmap = GMAP[half] if masking else (
            (0, 1, 2, 3) if half == 0 else (4, 5, 6, 7))
        qch = query[b].reshape(NCH, QCH, DM)
        q_sh = np.ascontiguousarray(
            np.concatenate([qch[g] for g in gmap], axis=0))
        in_maps.append({
            "q_in": q_sh, "k_in": key[b], "v_in": value[b],
            "wqt": wqt, "wkt": wkt, "wvt": wvt, "wot": wot,
            "bq2": bq2, "bk2": bk2, "bvr": bvr, "bor": bor,
            "msk": _build_masks(half) if masking else
                   np.zeros((4, 128, 1024), ml_dtypes.bfloat16),
        })

    return in_maps


def gather_out(core_outs, masking):
    out = np.empty((B, S, DM), np.float32)
    for c in range(N_CORES):
        b, half = c // 2, c % 2
        gmap = GMAP[half] if masking else (
            (0, 1, 2, 3) if half == 0 else (4, 5, 6, 7))
        o = np.asarray(core_outs[c]).reshape(4, QCH, DM)
        for s, g in enumerate(gmap):
            out[b, g * QCH:(g + 1) * QCH, :] = o[s]
    return out
